# revision 2
# baseline (speedup 1.0000x reference)
"""v4: per-slot matmul DHG kernel — no DRAM tables, no dma_gather.

Host lays out transposed per-slot features so the phase-A matmul output
lands directly in the phase-B (group-per-partition) layout.
"""
import numpy as np
import ml_dtypes
import concourse.bass as bass
import concourse.bacc as bacc
import concourse.tile as tile
from concourse import mybir

P = 128
NM = 5               # megatiles per core
KAP = 4              # edges per partition per megatile
GPP = KAP * 5        # 20 groups per partition
NIW = GPP * 8        # 160 slot-rows per partition
VT = NIW * P // P    # slots per partition-batch dim: VT cols = 160*128/128
NSL = NIW * P        # 20480 slots per megatile
NBT = 20             # batches per megatile (1024 slots each)
RS = 40              # record stride cols (q,k,v,G32,F2,pad3)
G8 = RS * 8          # group stride in record cols
TC = 40              # wcat cols
EPC = NM * P * KAP   # 2560 edges/core padded

bf = mybir.dt.bfloat16
f32 = mybir.dt.float32
MUL = mybir.AluOpType.mult
ADD = mybir.AluOpType.add
MAX = mybir.AluOpType.max
AF = mybir.ActivationFunctionType
X = mybir.AxisListType.X


def ap_of(t, off, dims):
    return bass.AP(tensor=t.tensor, offset=t.offset + off, ap=[list(t.ap[0])] + [list(d) for d in dims])


def build(n_cores=8, repeat=1):
    nc = bacc.Bacc("TRN2", target_bir_lowering=False, debug=False, num_devices=n_cores)
    fTC = nc.declare_dram_parameter("fTC", [NM, P, NSL], bf, isOutput=False)
    wcat_d = nc.declare_dram_parameter("wcat", [P, TC], bf, isOutput=False)
    cb_d = nc.declare_dram_parameter("consts", [P, 66], f32, isOutput=False)
    out_d = nc.declare_dram_parameter("out", [NM, P, KAP * 2], f32, isOutput=True)

    with tile.TileContext(nc) as tc:
        with tc.tile_pool(name="cons", bufs=1) as cons, \
             tc.tile_pool(name="pa", bufs=4) as pa, \
             tc.tile_pool(name="pap", bufs=4, space="PSUM") as pap, \
             tc.tile_pool(name="pb", bufs=2) as pb:
            wcat_t = cons.tile([P, TC], bf)
            nc.sync.dma_start(out=wcat_t[:], in_=wcat_d[:])
            cb_t = cons.tile([P, 66], f32)
            nc.sync.dma_start(out=cb_t[:], in_=cb_d[:])

            def phase_a(m):
                """DMA per-slot transposed feats, matmul vs wcat, records into SBUF."""
                rec = pb.tile([P, NIW * RS], bf, tag="rec")
                for b in range(NBT):
                    lhsT = pa.tile([P, 1024], bf, tag="lhsT")
                    nc.sync.dma_start(out=lhsT[:], in_=fTC[m][:, b * 1024:(b + 1) * 1024])
                    ps = pap.tile([P, 8 * TC], f32)
                    for c in range(8):
                        nc.tensor.matmul(out=ps[:, c * TC:(c + 1) * TC],
                                         lhsT=lhsT[:, c * P:(c + 1) * P], rhs=wcat_t[:],
                                         start=True, stop=True)
                    nc.scalar.copy(out=rec[:, b * 320:(b + 1) * 320], in_=ps[:])
                return rec

            def phase_b(m, rec):
                S = pb.tile([P, GPP * 64], f32, tag="S")
                nc.vector.tensor_tensor(
                    out=ap_of(S, 0, [(64, GPP), (8, 8), (1, 8)]),
                    in0=ap_of(rec, 0, [(G8, GPP), (RS, 8), (0, 8)]),
                    in1=ap_of(rec, 1, [(G8, GPP), (0, 8), (RS, 8)]), op=MUL)
                E = pb.tile([P, GPP * 64], bf, tag="E")
                nc.scalar.activation(out=E[:], in_=S[:], func=AF.Exp)
                nc.vector.memset(ap_of(E, 0, [(64, GPP), (9, 8)]), 0.0)
                rs = pb.tile([P, NIW], f32, tag="rs")
                nc.vector.tensor_reduce(out=rs[:], in_=ap_of(E, 0, [(64, GPP), (8, 8), (1, 8)]),
                                        axis=X, op=ADD)
                tv = pb.tile([P, GPP * 64], bf, tag="tv")
                nc.vector.tensor_tensor(
                    out=ap_of(tv, 0, [(64, GPP), (8, 8), (1, 8)]),
                    in0=ap_of(E, 0, [(64, GPP), (8, 8), (1, 8)]),
                    in1=ap_of(rec, 2, [(G8, GPP), (0, 8), (RS, 8)]), op=MUL)
                ts = pb.tile([P, NIW], f32, tag="ts")
                nc.vector.tensor_reduce(out=ts[:], in_=ap_of(tv, 0, [(64, GPP), (8, 8), (1, 8)]),
                                        axis=X, op=ADD)
                rv = pb.tile([P, NIW], f32, tag="rv")
                nc.vector.reciprocal(out=rv[:], in_=rs[:])
                td = pb.tile([P, NIW], f32, tag="td")
                nc.vector.tensor_tensor(out=td[:], in0=ts[:], in1=rv[:], op=MUL)
                dg = pb.tile([P, NIW], bf, tag="dg")
                nc.scalar.activation(out=dg[:], in_=td[:], func=AF.Tanh)
                prod = pb.tile([P, GPP * 256], bf, tag="prod")
                nc.vector.tensor_tensor(
                    out=ap_of(prod, 0, [(256, GPP), (32, 8), (1, 32)]),
                    in0=ap_of(rec, 3, [(G8, GPP), (RS, 8), (1, 32)]),
                    in1=ap_of(dg, 0, [(8, GPP), (1, 8), (0, 32)]), op=MUL)
                s1 = pb.tile([P, GPP * 128], bf, tag="s1")
                nc.vector.tensor_tensor(
                    out=ap_of(s1, 0, [(128, GPP), (32, 4), (1, 32)]),
                    in0=ap_of(prod, 0, [(256, GPP), (64, 4), (1, 32)]),
                    in1=ap_of(prod, 32, [(256, GPP), (64, 4), (1, 32)]), op=ADD)
                s2 = pb.tile([P, GPP * 64], bf, tag="s2")
                nc.vector.tensor_tensor(
                    out=ap_of(s2, 0, [(64, GPP), (32, 2), (1, 32)]),
                    in0=ap_of(s1, 0, [(128, GPP), (64, 2), (1, 32)]),
                    in1=ap_of(s1, 32, [(128, GPP), (64, 2), (1, 32)]), op=ADD)
                u = pb.tile([P, GPP * 32], f32, tag="u")
                nc.vector.tensor_tensor(
                    out=ap_of(u, 0, [(32, GPP), (1, 32)]),
                    in0=ap_of(s2, 0, [(64, GPP), (1, 32)]),
                    in1=ap_of(s2, 32, [(64, GPP), (1, 32)]), op=ADD)
                ub = pb.tile([P, GPP * 32], f32, tag="ub")
                nc.vector.tensor_tensor(out=ub[:], in0=u[:],
                                        in1=ap_of(cb_t, 0, [(0, GPP), (1, 32)]), op=ADD)
                rl = pb.tile([P, GPP * 32], f32, tag="rl")
                nc.vector.tensor_scalar(out=rl[:], in0=ub[:], scalar1=0.0, scalar2=None, op0=MAX)
                wm = pb.tile([P, GPP * 32], f32, tag="wm")
                nc.vector.tensor_tensor(out=wm[:], in0=rl[:],
                                        in1=ap_of(cb_t, 32, [(0, GPP), (1, 32)]), op=MUL)
                sc = pb.tile([P, GPP], f32, tag="sc")
                nc.vector.tensor_reduce(out=sc[:], in_=ap_of(wm, 0, [(32, GPP), (1, 32)]),
                                        axis=X, op=ADD)
                esc = pb.tile([P, GPP], f32, tag="esc")
                nc.scalar.activation(out=esc[:], in_=sc[:], func=AF.Exp)
                ssum = pb.tile([P, KAP], f32, tag="ssum")
                nc.vector.tensor_reduce(out=ssum[:], in_=ap_of(esc, 0, [(5, KAP), (1, 5)]),
                                        axis=X, op=ADD)
                sr = pb.tile([P, KAP], f32, tag="sr")
                nc.vector.reciprocal(out=sr[:], in_=ssum[:])
                av = pb.tile([P, GPP], f32, tag="av")
                nc.vector.tensor_tensor(out=av[:], in0=esc[:],
                                        in1=ap_of(sr, 0, [(1, KAP), (0, 5)]), op=MUL)
                prF = pb.tile([P, GPP * 16], bf, tag="prF")
                nc.vector.tensor_tensor(
                    out=ap_of(prF, 0, [(16, GPP), (2, 8), (1, 2)]),
                    in0=ap_of(rec, 35, [(G8, GPP), (RS, 8), (1, 2)]),
                    in1=ap_of(dg, 0, [(8, GPP), (1, 8), (0, 2)]), op=MUL)
                f1 = pb.tile([P, GPP * 8], bf, tag="f1")
                nc.vector.tensor_tensor(
                    out=ap_of(f1, 0, [(8, GPP), (2, 4), (1, 2)]),
                    in0=ap_of(prF, 0, [(16, GPP), (4, 4), (1, 2)]),
                    in1=ap_of(prF, 2, [(16, GPP), (4, 4), (1, 2)]), op=ADD)
                f2 = pb.tile([P, GPP * 4], bf, tag="f2")
                nc.vector.tensor_tensor(
                    out=ap_of(f2, 0, [(4, GPP), (2, 2), (1, 2)]),
                    in0=ap_of(f1, 0, [(8, GPP), (4, 2), (1, 2)]),
                    in1=ap_of(f1, 2, [(8, GPP), (4, 2), (1, 2)]), op=ADD)
                fs = pb.tile([P, GPP * 2], f32, tag="fs")
                nc.vector.tensor_tensor(
                    out=ap_of(fs, 0, [(2, GPP), (1, 2)]),
                    in0=ap_of(f2, 0, [(4, GPP), (1, 2)]),
                    in1=ap_of(f2, 2, [(4, GPP), (1, 2)]), op=ADD)
                ha = pb.tile([P, KAP * 10], f32, tag="ha")
                nc.vector.tensor_tensor(
                    out=ap_of(ha, 0, [(10, KAP), (5, 2), (1, 5)]),
                    in0=ap_of(fs, 0, [(10, KAP), (1, 2), (2, 5)]),
                    in1=ap_of(av, 0, [(5, KAP), (0, 2), (1, 5)]), op=MUL)
                lo = pb.tile([P, KAP * 2], f32, tag="lo")
                nc.vector.tensor_reduce(out=lo[:], in_=ap_of(ha, 0, [(10, KAP), (5, 2), (1, 5)]),
                                        axis=X, op=ADD)
                lb = pb.tile([P, KAP * 2], f32, tag="lb")
                nc.vector.tensor_tensor(out=lb[:], in0=lo[:],
                                        in1=ap_of(cb_t, 64, [(0, KAP), (1, 2)]), op=ADD)
                ov = pb.tile([P, KAP * 2], f32, tag="ov")
                nc.scalar.activation(out=ov[:], in_=lb[:], func=AF.Sigmoid)
                nc.sync.dma_start(out=out_d[m], in_=ov[:])

            for _rep in range(repeat):
                rec = phase_a(0)
                for m in range(NM):
                    rec_next = phase_a(m + 1) if m + 1 < NM else None
                    phase_b(m, rec)
                    rec = rec_next
    nc.compile()
    return nc


def host_prepare(feats, edge_members, adj_members, wq, wk, wv, W1, b1, W2, Wfc, bfc, n_cores=8):
    V, D = feats.shape
    E = edge_members.shape[0]
    epc_real = E // n_cores
    mem_all = np.concatenate([edge_members[:, None, :], adj_members], axis=1).astype(np.int64)  # [E,5,8]

    wcat = np.zeros((D, TC), np.float32)
    wcat[:, 0] = wq[:, 0]; wcat[:, 1] = wk[:, 0]; wcat[:, 2] = wv[:, 0]
    wcat[:, 3:35] = W1; wcat[:, 35:37] = Wfc
    wcat = wcat.astype(ml_dtypes.bfloat16)
    cb = np.zeros((P, 66), np.float32)
    cb[:, 0:32] = b1[None, :]; cb[:, 32:64] = W2[:, 0][None, :]; cb[:, 64:66] = bfc[None, :]
    feats_bf = np.asarray(feats, np.float32).astype(ml_dtypes.bfloat16)

    in_maps = []
    for c in range(n_cores):
        el = np.zeros((EPC,), np.int64)
        el[:epc_real] = np.arange(c * epc_real, (c + 1) * epc_real)
        mem = mem_all[el].reshape(NM, P, KAP, 5, 8)   # edge (m,p,k) = m*512 + p*4 + k
        fTC = np.zeros((NM, P, NSL), ml_dtypes.bfloat16)
        for m in range(NM):
            # slot (p, srow) at column srow*128+p; srow = k*40 + cc*8 + j
            V2 = mem[m].transpose(1, 2, 3, 0).reshape(NIW * P)   # [srow, p] flat
            fTC[m] = feats_bf[V2].T
        in_maps.append({"fTC": fTC, "wcat": wcat, "consts": cb})

    def unpack(results):
        outs = []
        for c in range(n_cores):
            o = results[c]["out"].reshape(NM, P, KAP, 2).reshape(EPC, 2)[:epc_real]
            outs.append(o)
        return np.concatenate(outs, axis=0)
    return in_maps, unpack


# ------------------------------------------------------------------
# Public entry point: kernel(**inputs) -> [20000, 2] float32
# ------------------------------------------------------------------
from concourse.bass_utils import run_bass_kernel_spmd

_CACHED_NC = None

def kernel(feats, edge_members, adj_members, ids, epoch,
           wq, bq, wk, bk, wv, bv, W1, b1, W2, b2, Wfc, bfc):
    """DHGLayerV1 forward on 8 NeuronCores.

    Strategy: edges sharded across 8 cores (2500 each), 5 megatiles per
    core. Host lays out bf16 feats transposed per SLOT (one column per
    group-member occurrence, ordered so the phase-A matmul against
    [wq|wk|wv|W1|Wfc] lands records (q,k,v,G32,F2) directly in the
    group-per-partition phase-B layout). Phase B (masked softmax over
    K=8, tanh gate, dg-weighted G32/F2 sums, relu-MLP score, softmax
    over 5 candidates, sigmoid head) runs on DVE/ACT. b2 is dropped
    (softmax-invariant); bq/bk/bv asserted zero; b1/bfc applied exactly.
    """
    global _CACHED_NC
    feats = np.asarray(feats, dtype=np.float32)
    edge_members = np.asarray(edge_members)
    adj_members = np.asarray(adj_members)
    wq = np.asarray(wq, np.float32); wk = np.asarray(wk, np.float32)
    wv = np.asarray(wv, np.float32); W1 = np.asarray(W1, np.float32)
    b1 = np.asarray(b1, np.float32); W2 = np.asarray(W2, np.float32)
    Wfc = np.asarray(Wfc, np.float32); bfc = np.asarray(bfc, np.float32)
    assert np.all(np.asarray(bq) == 0) and np.all(np.asarray(bk) == 0) \
        and np.all(np.asarray(bv) == 0), "nonzero q/k/v biases unsupported"

    if _CACHED_NC is None:
        _CACHED_NC = build(n_cores=8)
    nc = _CACHED_NC
    in_maps, unpack = host_prepare(feats, edge_members, adj_members,
                                   wq, wk, wv, W1, b1, W2, Wfc, bfc, n_cores=8)
    res = run_bass_kernel_spmd(nc, in_maps, core_ids=list(range(8)))
    return unpack(res.results).astype(np.float32)


# revision 13
# speedup vs baseline: 1.2541x; 1.2541x over previous
"""v6: per-slot bf16 matmul DHG kernel — no DRAM tables, no dma_gather.

Host lays out transposed per-slot features (fp8) so the phase-A matmul
output lands directly in the phase-B (group-per-partition) layout.
"""
import numpy as np
import ml_dtypes
import concourse.bass as bass
import concourse.bacc as bacc
import concourse.tile as tile
from concourse import mybir

P = 128
NM = 5               # megatiles per core
KAP = 4              # edges per partition per megatile
GPP = KAP * 5        # 20 groups per partition
NIW = GPP * 8        # 160 slot-rows per partition
NSL = NIW * P        # 20480 slots per megatile
BSZ = 2048           # slots per DMA batch
NBT = NSL // BSZ     # 10 batches per megatile
RS = 40              # record stride cols (q,k,v,G32,F2,pad3)
G8 = RS * 8          # group stride in record cols
TC = 40              # wcat cols
EPC = NM * P * KAP   # 2560 edges/core padded

bf = mybir.dt.bfloat16
f32 = mybir.dt.float32
f8 = mybir.dt.float8e4
MUL = mybir.AluOpType.mult
ADD = mybir.AluOpType.add
MAX = mybir.AluOpType.max
AF = mybir.ActivationFunctionType
X = mybir.AxisListType.X


def ap_of(t, off, dims):
    return bass.AP(tensor=t.tensor, offset=t.offset + off, ap=[list(t.ap[0])] + [list(d) for d in dims])


def build(n_cores=8, repeat=1, mode="full", loop_n=None, fdt=None):
    FDT = fdt or bf
    nc = bacc.Bacc("TRN2", target_bir_lowering=False, debug=False, num_devices=n_cores)
    fTC = nc.declare_dram_parameter("fTC", [NM, NBT, P, BSZ], FDT, isOutput=False)
    wcat_d = nc.declare_dram_parameter("wcat", [P, TC], bf, isOutput=False)
    cb_d = nc.declare_dram_parameter("consts", [P, 66], f32, isOutput=False)
    cbh_d = nc.declare_dram_parameter("constsh", [P, 64], bf, isOutput=False)
    out_d = nc.declare_dram_parameter("out", [NM, P, KAP * 2], f32, isOutput=True)

    with tile.TileContext(nc) as tc:
        with tc.tile_pool(name="cons", bufs=1) as cons, \
             tc.tile_pool(name="pa", bufs=4) as pa, \
             tc.tile_pool(name="pap", bufs=4, space="PSUM") as pap, \
             tc.tile_pool(name="pb", bufs=2) as pb:
            wcat_t = cons.tile([P, TC], bf)
            nc.sync.dma_start(out=wcat_t[:], in_=wcat_d[:])
            cb_t = cons.tile([P, 66], f32)
            nc.sync.dma_start(out=cb_t[:], in_=cb_d[:])
            cbh_t = cons.tile([P, 64], bf)
            nc.sync.dma_start(out=cbh_t[:], in_=cbh_d[:])

            def phase_a(m):
                """DMA per-slot transposed feats, matmul vs wcat, records into SBUF."""
                rec = pb.tile([P, NIW * RS], bf, tag="rec")
                for b in range(NBT):
                    lhsT = pa.tile([P, BSZ], FDT, tag="lhsT")
                    nc.sync.dma_start(out=lhsT[:], in_=fTC[m][b])
                    if mode == "dmaonly":
                        continue
                    for h in range(2):
                        ps = pap.tile([P, 8 * TC], f32)
                        for c in range(8):
                            nc.tensor.matmul(out=ps[:, c * TC:(c + 1) * TC],
                                             lhsT=lhsT[:, (h * 8 + c) * P:(h * 8 + c + 1) * P],
                                             rhs=wcat_t[:], start=True, stop=True)
                        if mode == "nocopy":
                            continue
                        nc.scalar.copy(out=rec[:, (b * 2 + h) * 320:(b * 2 + h + 1) * 320],
                                       in_=ps[:])
                return rec

            def phase_b(m, rec):
                S = pb.tile([P, GPP * 64], bf, tag="S")
                nc.vector.tensor_tensor(
                    out=ap_of(S, 0, [(64, GPP), (8, 8), (1, 8)]),
                    in0=ap_of(rec, 0, [(G8, GPP), (RS, 8), (0, 8)]),
                    in1=ap_of(rec, 1, [(G8, GPP), (0, 8), (RS, 8)]), op=MUL)
                E = pb.tile([P, GPP * 64], bf, tag="E")
                nc.scalar.activation(out=E[:], in_=S[:], func=AF.Exp)
                nc.scalar.activation(out=ap_of(E, 0, [(64, GPP), (9, 8)]),
                                     in_=ap_of(E, 0, [(64, GPP), (9, 8)]),
                                     func=AF.Copy, scale=0.0)
                # row-sum of E over l: 3-level pairwise tree (2x DVE mode)
                r4 = pb.tile([P, GPP * 32], bf, tag="r4")
                nc.vector.tensor_tensor(
                    out=ap_of(r4, 0, [(32, GPP), (4, 8), (1, 4)]),
                    in0=ap_of(E, 0, [(64, GPP), (8, 8), (1, 4)]),
                    in1=ap_of(E, 4, [(64, GPP), (8, 8), (1, 4)]), op=ADD)
                r2 = pb.tile([P, GPP * 16], bf, tag="r2")
                nc.vector.tensor_tensor(
                    out=ap_of(r2, 0, [(16, GPP), (2, 8), (1, 2)]),
                    in0=ap_of(r4, 0, [(32, GPP), (4, 8), (1, 2)]),
                    in1=ap_of(r4, 2, [(32, GPP), (4, 8), (1, 2)]), op=ADD)
                rs = pb.tile([P, NIW], bf, tag="rs")
                nc.vector.tensor_tensor(
                    out=ap_of(rs, 0, [(8, GPP), (1, 8)]),
                    in0=ap_of(r2, 0, [(16, GPP), (2, 8)]),
                    in1=ap_of(r2, 1, [(16, GPP), (2, 8)]), op=ADD)
                tv = pb.tile([P, GPP * 64], bf, tag="tv")
                nc.vector.tensor_tensor(
                    out=ap_of(tv, 0, [(64, GPP), (8, 8), (1, 8)]),
                    in0=ap_of(E, 0, [(64, GPP), (8, 8), (1, 8)]),
                    in1=ap_of(rec, 2, [(G8, GPP), (0, 8), (RS, 8)]), op=MUL)
                t4 = pb.tile([P, GPP * 32], bf, tag="t4")
                nc.vector.tensor_tensor(
                    out=ap_of(t4, 0, [(32, GPP), (4, 8), (1, 4)]),
                    in0=ap_of(tv, 0, [(64, GPP), (8, 8), (1, 4)]),
                    in1=ap_of(tv, 4, [(64, GPP), (8, 8), (1, 4)]), op=ADD)
                t2 = pb.tile([P, GPP * 16], bf, tag="t2")
                nc.vector.tensor_tensor(
                    out=ap_of(t2, 0, [(16, GPP), (2, 8), (1, 2)]),
                    in0=ap_of(t4, 0, [(32, GPP), (4, 8), (1, 2)]),
                    in1=ap_of(t4, 2, [(32, GPP), (4, 8), (1, 2)]), op=ADD)
                ts = pb.tile([P, NIW], bf, tag="ts")
                nc.vector.tensor_tensor(
                    out=ap_of(ts, 0, [(8, GPP), (1, 8)]),
                    in0=ap_of(t2, 0, [(16, GPP), (2, 8)]),
                    in1=ap_of(t2, 1, [(16, GPP), (2, 8)]), op=ADD)
                rv = pb.tile([P, NIW], f32, tag="rv")
                nc.vector.reciprocal(out=rv[:], in_=rs[:])
                td = pb.tile([P, NIW], f32, tag="td")
                nc.vector.tensor_tensor(out=td[:], in0=ts[:], in1=rv[:], op=MUL)
                dg = pb.tile([P, NIW], bf, tag="dg")
                nc.scalar.activation(out=dg[:], in_=td[:], func=AF.Tanh)
                prod = pb.tile([P, GPP * 256], bf, tag="prod")
                nc.vector.tensor_tensor(
                    out=ap_of(prod, 0, [(256, GPP), (32, 8), (1, 32)]),
                    in0=ap_of(rec, 3, [(G8, GPP), (RS, 8), (1, 32)]),
                    in1=ap_of(dg, 0, [(8, GPP), (1, 8), (0, 32)]), op=MUL)
                s1 = pb.tile([P, GPP * 128], bf, tag="s1")
                nc.vector.tensor_tensor(
                    out=ap_of(s1, 0, [(128, GPP), (32, 4), (1, 32)]),
                    in0=ap_of(prod, 0, [(256, GPP), (64, 4), (1, 32)]),
                    in1=ap_of(prod, 32, [(256, GPP), (64, 4), (1, 32)]), op=ADD)
                s2 = pb.tile([P, GPP * 64], bf, tag="s2")
                nc.vector.tensor_tensor(
                    out=ap_of(s2, 0, [(64, GPP), (32, 2), (1, 32)]),
                    in0=ap_of(s1, 0, [(128, GPP), (64, 2), (1, 32)]),
                    in1=ap_of(s1, 32, [(128, GPP), (64, 2), (1, 32)]), op=ADD)
                u = pb.tile([P, GPP * 32], bf, tag="u")
                nc.vector.tensor_tensor(
                    out=ap_of(u, 0, [(32, GPP), (1, 32)]),
                    in0=ap_of(s2, 0, [(64, GPP), (1, 32)]),
                    in1=ap_of(s2, 32, [(64, GPP), (1, 32)]), op=ADD)
                ub = pb.tile([P, GPP * 32], bf, tag="ub")
                nc.vector.tensor_tensor(out=ub[:], in0=u[:],
                                        in1=ap_of(cbh_t, 0, [(0, GPP), (1, 32)]), op=ADD)
                rl = pb.tile([P, GPP * 32], bf, tag="rl")
                nc.scalar.activation(out=rl[:], in_=ub[:], func=AF.Relu)
                wm = pb.tile([P, GPP * 32], bf, tag="wm")
                nc.vector.tensor_tensor(out=wm[:], in0=rl[:],
                                        in1=ap_of(cbh_t, 32, [(0, GPP), (1, 32)]), op=MUL)
                sc = pb.tile([P, GPP], f32, tag="sc")
                nc.vector.tensor_reduce(out=sc[:], in_=ap_of(wm, 0, [(32, GPP), (1, 32)]),
                                        axis=X, op=ADD)
                esc = pb.tile([P, GPP], f32, tag="esc")
                nc.scalar.activation(out=esc[:], in_=sc[:], func=AF.Exp)
                ssum = pb.tile([P, KAP], f32, tag="ssum")
                nc.vector.tensor_reduce(out=ssum[:], in_=ap_of(esc, 0, [(5, KAP), (1, 5)]),
                                        axis=X, op=ADD)
                sr = pb.tile([P, KAP], f32, tag="sr")
                nc.vector.reciprocal(out=sr[:], in_=ssum[:])
                av = pb.tile([P, GPP], f32, tag="av")
                nc.vector.tensor_tensor(out=av[:], in0=esc[:],
                                        in1=ap_of(sr, 0, [(1, KAP), (0, 5)]), op=MUL)
                prF = pb.tile([P, GPP * 16], bf, tag="prF")
                nc.vector.tensor_tensor(
                    out=ap_of(prF, 0, [(16, GPP), (2, 8), (1, 2)]),
                    in0=ap_of(rec, 35, [(G8, GPP), (RS, 8), (1, 2)]),
                    in1=ap_of(dg, 0, [(8, GPP), (1, 8), (0, 2)]), op=MUL)
                f1 = pb.tile([P, GPP * 8], bf, tag="f1")
                nc.vector.tensor_tensor(
                    out=ap_of(f1, 0, [(8, GPP), (2, 4), (1, 2)]),
                    in0=ap_of(prF, 0, [(16, GPP), (4, 4), (1, 2)]),
                    in1=ap_of(prF, 2, [(16, GPP), (4, 4), (1, 2)]), op=ADD)
                f2 = pb.tile([P, GPP * 4], bf, tag="f2")
                nc.vector.tensor_tensor(
                    out=ap_of(f2, 0, [(4, GPP), (2, 2), (1, 2)]),
                    in0=ap_of(f1, 0, [(8, GPP), (4, 2), (1, 2)]),
                    in1=ap_of(f1, 2, [(8, GPP), (4, 2), (1, 2)]), op=ADD)
                fs = pb.tile([P, GPP * 2], f32, tag="fs")
                nc.vector.tensor_tensor(
                    out=ap_of(fs, 0, [(2, GPP), (1, 2)]),
                    in0=ap_of(f2, 0, [(4, GPP), (1, 2)]),
                    in1=ap_of(f2, 2, [(4, GPP), (1, 2)]), op=ADD)
                ha = pb.tile([P, KAP * 10], f32, tag="ha")
                nc.vector.tensor_tensor(
                    out=ap_of(ha, 0, [(10, KAP), (5, 2), (1, 5)]),
                    in0=ap_of(fs, 0, [(10, KAP), (1, 2), (2, 5)]),
                    in1=ap_of(av, 0, [(5, KAP), (0, 2), (1, 5)]), op=MUL)
                lo = pb.tile([P, KAP * 2], f32, tag="lo")
                nc.vector.tensor_reduce(out=lo[:], in_=ap_of(ha, 0, [(10, KAP), (5, 2), (1, 5)]),
                                        axis=X, op=ADD)
                lb = pb.tile([P, KAP * 2], f32, tag="lb")
                nc.vector.tensor_tensor(out=lb[:], in0=lo[:],
                                        in1=ap_of(cb_t, 64, [(0, KAP), (1, 2)]), op=ADD)
                # sigmoid(x) = 0.5*tanh(0.5x)+0.5 — keeps ACT in the
                # exp_and_others table set (Sigmoid would force a ~2.7us
                # table swap per megatile)
                th = pb.tile([P, KAP * 2], f32, tag="th")
                nc.scalar.activation(out=th[:], in_=lb[:], func=AF.Tanh, scale=0.5)
                ov = pb.tile([P, KAP * 2], f32, tag="ov")
                nc.vector.tensor_scalar(out=ov[:], in0=th[:], scalar1=0.5, scalar2=0.5,
                                        op0=MUL, op1=ADD)
                nc.sync.dma_start(out=out_d[m], in_=ov[:])

            def one_pass():
                rec = phase_a(0)
                for m in range(NM):
                    rec_next = phase_a(m + 1) if m + 1 < NM else None
                    if mode == "full":
                        phase_b(m, rec)
                    rec = rec_next

            if loop_n is not None:
                with tc.For_i(0, loop_n):
                    for _rep in range(repeat):
                        one_pass()
            else:
                for _rep in range(repeat):
                    one_pass()
    nc.compile()
    return nc


HOST_FDT = ml_dtypes.bfloat16


def host_prepare(feats, edge_members, adj_members, wq, wk, wv, W1, b1, W2, Wfc, bfc, n_cores=8):
    V, D = feats.shape
    E = edge_members.shape[0]
    epc_real = E // n_cores
    mem_all = np.concatenate([edge_members[:, None, :], adj_members], axis=1).astype(np.int64)  # [E,5,8]

    wcat = np.zeros((D, TC), np.float32)
    wcat[:, 0] = wq[:, 0]; wcat[:, 1] = wk[:, 0]; wcat[:, 2] = wv[:, 0]
    wcat[:, 3:35] = W1; wcat[:, 35:37] = Wfc
    wcat = wcat.astype(ml_dtypes.bfloat16)
    cb = np.zeros((P, 66), np.float32)
    cb[:, 0:32] = b1[None, :]; cb[:, 32:64] = W2[:, 0][None, :]; cb[:, 64:66] = bfc[None, :]
    cbh = cb[:, :64].astype(ml_dtypes.bfloat16)
    feats_f8 = np.asarray(feats, np.float32).astype(HOST_FDT)

    in_maps = []
    for c in range(n_cores):
        el = np.zeros((EPC,), np.int64)
        el[:epc_real] = np.arange(c * epc_real, (c + 1) * epc_real)
        mem = mem_all[el].reshape(NM, P, KAP, 5, 8)   # edge (m,p,k) = m*512 + p*4 + k
        fTC = np.zeros((NM, NBT, P, BSZ), HOST_FDT)
        for m in range(NM):
            # slot (p, srow) at column srow*128+p; srow = k*40 + cc*8 + j
            V2 = mem[m].transpose(1, 2, 3, 0).reshape(NIW * P)   # [srow, p] flat
            fTC[m] = feats_f8[V2].T.reshape(P, NBT, BSZ).transpose(1, 0, 2)
        in_maps.append({"fTC": fTC, "wcat": wcat, "consts": cb, "constsh": cbh})

    def unpack(results):
        outs = []
        for c in range(n_cores):
            o = results[c]["out"].reshape(NM, P, KAP, 2).reshape(EPC, 2)[:epc_real]
            outs.append(o)
        return np.concatenate(outs, axis=0)
    return in_maps, unpack


# ------------------------------------------------------------------
# Public entry point: kernel(**inputs) -> [20000, 2] float32
# ------------------------------------------------------------------
from concourse.bass_utils import run_bass_kernel_spmd

_CACHED_NC = None

def kernel(feats, edge_members, adj_members, ids, epoch,
           wq, bq, wk, bk, wv, bv, W1, b1, W2, b2, Wfc, bfc):
    """DHGLayerV1 forward on 8 NeuronCores.

    Strategy: edges sharded across 8 cores (2500 each), 5 megatiles per
    core. Host lays out bf16 feats transposed per SLOT (one column per
    group-member occurrence, ordered so the phase-A matmul against
    bf16 [wq|wk|wv|W1|Wfc] lands records (q,k,v,G32,F2) directly in the
    group-per-partition phase-B layout). Phase B (masked softmax over
    K=8, tanh gate, dg-weighted G32/F2 sums, relu-MLP score, softmax
    over 5 candidates, sigmoid head) runs on DVE/ACT. b2 is dropped
    (softmax-invariant); bq/bk/bv asserted zero; b1/bfc applied exactly.
    """
    global _CACHED_NC
    feats = np.asarray(feats, dtype=np.float32)
    edge_members = np.asarray(edge_members)
    adj_members = np.asarray(adj_members)
    wq = np.asarray(wq, np.float32); wk = np.asarray(wk, np.float32)
    wv = np.asarray(wv, np.float32); W1 = np.asarray(W1, np.float32)
    b1 = np.asarray(b1, np.float32); W2 = np.asarray(W2, np.float32)
    Wfc = np.asarray(Wfc, np.float32); bfc = np.asarray(bfc, np.float32)
    assert np.all(np.asarray(bq) == 0) and np.all(np.asarray(bk) == 0) \
        and np.all(np.asarray(bv) == 0), "nonzero q/k/v biases unsupported"

    if _CACHED_NC is None:
        _CACHED_NC = build(n_cores=8)
    nc = _CACHED_NC
    in_maps, unpack = host_prepare(feats, edge_members, adj_members,
                                   wq, wk, wv, W1, b1, W2, Wfc, bfc, n_cores=8)
    res = run_bass_kernel_spmd(nc, in_maps, core_ids=list(range(8)))
    return unpack(res.results).astype(np.float32)
